# revision 1
# baseline (speedup 1.0000x reference)
"""Banded multi-head attention (B=2, L=1024, D=1024, H=16, band W=64) on 8
Trainium2 NeuronCores.

Sharding: core = (batch b, head-group g) with 2 batches x 4 head groups of 4
heads each.  Each core computes q/k/v projections for its group (f32r
matmuls), the banded attention for its 4 heads, and a partial output
projection through its slice of Wo.  Host sums the 4 partial outputs per
batch.

Device schedule notes:
- All matmul operands are pre-transposed on host so every DMA is contiguous:
  xT [din, L], wqT/wkT [din, dq] (lhsT), wvT [din, dv] (rhs), woT [dv, dout].
- Input DMAs stream K-chunks (weights + first token-half of x) so the first
  half of every projection can start while the rest streams in; attention for
  the first two query tiles is emitted before the second-half projections so
  it fills the TensorEngine under the DMA tail.
- Scores are computed transposed, S^T[span_key, query], per head pair into a
  [128, 512] PSUM tile, 3 chunks of 128 keys per 256-query tile; the key axis
  is padded left by 128 (65 zeros + 63 learned cache entries) so every chunk
  is a full 128 partitions.  Matmul operands always start at partition 0
  (base-64 operands wedge the device).
- Band mask (+1/sqrt(dh) scale) is one scalar_tensor_tensor per pair-chunk;
  exp on the scalar engine writes f32r attention weights.  Exp and Ln are
  pinned to the one act-func set containing both, loaded once (alternating
  table loads wedge the device).
- V is stored token-major with a ones-column per head; attn @ V then yields
  o^T[dv, query] plus the softmax denominator row.  1/denom = exp(-ln d) on
  the scalar engine, broadcast across partitions with a K=1 f32r matmul.
"""
import numpy as np

import concourse.bacc as bacc
import concourse.mybir as mybir
import concourse.tile as tile
from concourse import bass_utils

B, L, D, H, W = 2, 1024, 1024, 16, 64
DH = D // H           # 64
G = 4                 # head groups
HPG = H // G          # 4 heads per group
DG = D // G           # 256 dims per group
NCORES = 8

F32 = mybir.dt.float32
F32R = mybir.dt.float32r
NEG = -1.0e30
EXPF = mybir.ActivationFunctionType.Exp
LNF = mybir.ActivationFunctionType.Ln


def _pin_exp_ln_table(arch: str):
    """Resolve Copy/Exp/Ln only to the natural_log_exp_and_others act-func
    set so exactly one table load is emitted (alternating per-function table
    swaps wedge the device)."""
    import concourse.hw_specs as hw_specs
    tables = hw_specs.get_activation_tables(arch)   # cached, mutable
    drop = {EXPF, LNF, mybir.ActivationFunctionType.Copy,
            mybir.ActivationFunctionType.Identity}
    assert "natural_log_exp_and_others" in tables
    for name, funcs in tables.items():
        if name != "natural_log_exp_and_others":
            funcs -= drop


def build(repeat: int = 1, variant: str = "full", loop_n: int = 0):
    """Build + compile the per-core Bass program.  loop_n > 0 wraps the body
    in a device-side For_i executing it loop_n times (for HW timing)."""
    nc = bacc.Bacc("TRN2", target_bir_lowering=False, debug=False)
    _pin_exp_ln_table(nc.m.arch)

    xT = nc.dram_tensor("xT", [D, L], F32R, kind="ExternalInput")
    wqT = nc.dram_tensor("wqT", [D, DG], F32R, kind="ExternalInput")
    wkT = nc.dram_tensor("wkT", [D, DG], F32R, kind="ExternalInput")
    wvT = nc.dram_tensor("wvT", [D, DG], F32R, kind="ExternalInput")
    woT = nc.dram_tensor("woT", [DG, D], F32R, kind="ExternalInput")
    kc = nc.dram_tensor("kc", [DG, 128], F32R, kind="ExternalInput")
    vc = nc.dram_tensor("vc", [128, HPG * (DH + 1)], F32R, kind="ExternalInput")
    onesr = nc.dram_tensor("onesr", [128, 32], F32R, kind="ExternalInput")
    onesf = nc.dram_tensor("onesf", [1, 64], F32, kind="ExternalInput")
    maskd = nc.dram_tensor("mask", [3, 128, 512], F32, kind="ExternalInput")
    y = nc.dram_tensor("y", [L, D], F32, kind="ExternalOutput")

    VSLOT = DH + 1                    # 65 cols per (slot, head)
    VROW = HPG * VSLOT                # 260 cols per slot
    NSLOT = L // 128 + 1              # 9 slots (slot 0 = cache block)

    with tile.TileContext(nc) as tc:
        with tc.tile_pool(name="res", bufs=1) as res, \
             tc.tile_pool(name="epool", bufs=9) as epool, \
             tc.tile_pool(name="rcpool", bufs=4) as rcpool, \
             tc.tile_pool(name="ypool", bufs=4) as ypool, \
             tc.tile_pool(name="ps", bufs=8, space="PSUM") as psp:

            # ---- resident SBUF tensors ----------------------------------
            xk = [res.tile([128, L], F32R, tag=f"xk{k}", name=f"xk{k}")
                  for k in range(8)]
            wqk = [res.tile([128, DG], F32R, tag=f"wq{k}", name=f"wq{k}")
                   for k in range(8)]
            wkk = [res.tile([128, DG], F32R, tag=f"wk{k}", name=f"wk{k}")
                   for k in range(8)]
            wvk = [res.tile([128, DG], F32R, tag=f"wv{k}", name=f"wv{k}")
                   for k in range(8)]
            wo_sb = [res.tile([128, D], F32R, tag=f"wo{m}", name=f"wo{m}")
                     for m in range(2)]
            qT = [res.tile([64, L], F32R, tag=f"qT{h}", name=f"qT{h}")
                  for h in range(4)]
            kT = [res.tile([64, 128 + L], F32R, tag=f"kT{h}", name=f"kT{h}")
                  for h in range(4)]
            v_sb = res.tile([128, NSLOT * VROW], F32R, tag="v", name="v_sb")
            mask_sb = res.tile([128, 3 * 512], F32, tag="mask", name="mask_sb")
            oT = [res.tile([128, L], F32R, tag=f"oT{m}", name=f"oT{m}")
                  for m in range(2)]
            ones_sb = res.tile([1, 64], F32, tag="ones", name="ones_sb")

            def emit_qk_group(wt, dst, off, m, n):
                pt = psp.tile([128, 512], F32, tag="ps", name="pj")
                for k in range(8):
                    nc.tensor.matmul(
                        pt[:],
                        wt[k][:, m * 128:(m + 1) * 128],
                        xk[k][:, n * 512:(n + 1) * 512],
                        start=(k == 0), stop=(k == 7),
                    )
                for hh in range(2):
                    nc.scalar.copy(
                        dst[2 * m + hh][:, off + n * 512: off + n * 512 + 512],
                        pt[hh * 64:(hh + 1) * 64, :])

            def emit_v(t):
                pv = psp.tile([128, 512], F32, tag="ps", name="pjv")
                for k in range(8):
                    nc.tensor.matmul(
                        pv[:, 0:DG],
                        xk[k][:, t * 128:(t + 1) * 128],
                        wvk[k][:],
                        start=(k == 0), stop=(k == 7),
                    )
                si = t + 1
                dst = v_sb[:, si * VROW:(si + 1) * VROW].rearrange(
                    "p (h c) -> p h c", c=VSLOT)[:, :, 0:DH]
                nc.vector.tensor_copy(
                    dst, pv[:, 0:DG].rearrange("p (h c) -> p h c", c=DH))

            def emit_attention(ti):
                t0 = ti * 256
                for m in range(2):       # head pairs
                    es = []
                    for s in range(3):   # 128-key span chunks
                        st = psp.tile([128, 512], F32, tag="ps", name="st")
                        for hh in range(2):
                            h = 2 * m + hh
                            nc.tensor.matmul(
                                st[:, hh * 256:(hh + 1) * 256],
                                kT[h][:, t0 + s * 128: t0 + s * 128 + 128],
                                qT[h][:, t0:t0 + 256],
                                start=True, stop=True,
                            )
                        nc.vector.scalar_tensor_tensor(
                            st[:], st[:], float(DH) ** -0.5,
                            mask_sb[:, s * 512:(s + 1) * 512],
                            mybir.AluOpType.mult, mybir.AluOpType.add,
                        )
                        e = epool.tile([128, 512], F32R, tag="e", name="e")
                        nc.scalar.activation(e[:], st[:], EXPF)
                        es.append(e)
                    ops = []
                    rc2 = rcpool.tile([1, 512], F32, tag="rc", name="rc2")
                    for hh in range(2):
                        h = 2 * m + hh
                        op = psp.tile([128, 512], F32, tag="ps", name="o")
                        for s in range(3):
                            si = 2 * ti + s
                            nc.tensor.matmul(
                                op[0:65, 0:256],
                                v_sb[:, si * VROW + h * VSLOT:
                                     si * VROW + h * VSLOT + VSLOT],
                                es[s][:, hh * 256:(hh + 1) * 256],
                                start=(s == 0), stop=(s == 2),
                            )
                        ops.append(op)
                        # 1/denom as exp(-ln d); both heads' rows share one
                        # broadcast matmul + exp below
                        nc.scalar.activation(
                            rc2[0:1, hh * 256:(hh + 1) * 256],
                            op[64:65, 0:256], LNF)
                    bcp = psp.tile([64, 512], F32, tag="ps", name="bcp")
                    nc.tensor.matmul(bcp[:], ones_sb[:], rc2[:],
                                     start=True, stop=True)
                    bc = rcpool.tile([64, 512], F32, tag="bc", name="bc")
                    nc.scalar.activation(bc[:], bcp[:], EXPF, scale=-1.0)
                    for hh in range(2):
                        oT_dst = oT[m][hh * 64:(hh + 1) * 64, t0:t0 + 256]
                        nc.vector.tensor_mul(
                            oT_dst, ops[hh][0:64, 0:256],
                            bc[:, hh * 256:(hh + 1) * 256])

            def emit_oproj(t):
                for n2 in range(2):
                    yp = psp.tile([128, 512], F32, tag="ps", name="yp")
                    for m in range(2):
                        nc.tensor.matmul(
                            yp[:],
                            oT[m][:, t * 128:(t + 1) * 128],
                            wo_sb[m][:, n2 * 512:(n2 + 1) * 512],
                            start=(m == 0), stop=(m == 1),
                        )
                    ysb = ypool.tile([128, 512], F32, tag="y", name="ysb")
                    if t % 2 == 0:
                        nc.scalar.copy(ysb[:], yp[:])
                    else:
                        nc.vector.tensor_copy(ysb[:], yp[:])
                    nc.sync.dma_start(
                        y.ap()[t * 128:(t + 1) * 128,
                               n2 * 512:(n2 + 1) * 512],
                        ysb[:])

            import contextlib

            def rep_ctx():
                if loop_n:
                    return tc.For_i(0, loop_n, 1,
                                    hint_engines=(mybir.EngineType.PE,
                                                  mybir.EngineType.Activation,
                                                  mybir.EngineType.DVE,
                                                  mybir.EngineType.SP))
                return contextlib.nullcontext()

            with rep_ctx():
              for rep in range(repeat):
                  do_in = variant != "empty"
                  do_compute = variant not in ("empty", "dmaonly")

                  # ---- input DMAs: K-chunk streaming, x first-half first ---
                  if do_in:
                      for k in range(8):
                          nc.sync.dma_start(wqk[k][:],
                                            wqT.ap()[k * 128:(k + 1) * 128, :])
                          nc.sync.dma_start(wkk[k][:],
                                            wkT.ap()[k * 128:(k + 1) * 128, :])
                          nc.sync.dma_start(wvk[k][:],
                                            wvT.ap()[k * 128:(k + 1) * 128, :])
                          nc.sync.dma_start(
                              xk[k][:, 0:512],
                              xT.ap()[k * 128:(k + 1) * 128, 0:512])
                          if k == 0:
                              for h in range(4):
                                  nc.sync.dma_start(
                                      kT[h][:, 0:128],
                                      kc.ap()[h * 64:(h + 1) * 64, :])
                              nc.sync.dma_start(v_sb[:, 0:VROW], vc.ap())
                              ones_cols = v_sb[:, VROW:].rearrange(
                                  "p (n c) -> p n c", c=VSLOT)[:, :, DH:DH + 1]
                              nc.sync.dma_start(
                                  ones_cols, onesr.ap()[:, 0:32].unsqueeze(2))
                              nc.sync.dma_start(ones_sb[:], onesf.ap())
                      nc.sync.dma_start(
                          mask_sb[:].rearrange("p (s n) -> p s n", s=3),
                          maskd.ap().rearrange("s p n -> p s n"),
                      )
                      for m in range(2):
                          nc.sync.dma_start(wo_sb[m][:],
                                            woT.ap()[m * 128:(m + 1) * 128, :])
                      for k in range(8):
                          nc.sync.dma_start(
                              xk[k][:, 512:1024],
                              xT.ap()[k * 128:(k + 1) * 128, 512:1024])

                  if not do_compute:
                      for t in range(8):
                          if variant == "empty":
                              nc.sync.dma_start(
                                  y.ap()[t * 128:(t + 1) * 128, :],
                                  xT.ap()[t * 128:(t + 1) * 128, :].bitcast(F32))
                          else:
                              nc.sync.dma_start(
                                  y.ap()[t * 128:(t + 1) * 128, :],
                                  xk[t][:].bitcast(F32))
                      continue

                  # ---- first half: projections, attention 0-1, oproj 0-3 --
                  for m in range(2):
                      emit_qk_group(wqk, qT, 0, m, 0)
                      emit_qk_group(wkk, kT, 128, m, 0)
                  emit_v(0)
                  emit_v(1)
                  emit_v(2)
                  emit_attention(0)
                  emit_v(3)
                  emit_attention(1)
                  emit_oproj(0)
                  emit_oproj(1)
                  # ---- second half -----------------------------------------
                  for m in range(2):
                      emit_qk_group(wqk, qT, 0, m, 1)
                      emit_qk_group(wkk, kT, 128, m, 1)
                  emit_oproj(2)
                  emit_oproj(3)
                  emit_v(4)
                  emit_v(5)
                  emit_v(6)
                  emit_attention(2)
                  emit_v(7)
                  emit_attention(3)
                  for t in range(4, 8):
                      emit_oproj(t)

    nc.compile()
    return nc


def make_mask() -> np.ndarray:
    """[3, 128, 512] additive mask (0 in band, NEG outside), doubled for the
    two heads sharing one 512-wide score tile.  Chunk s, row r (key index
    t0 + s*128 + r - 128), query col i valid iff the key is within the
    64-wide causal band of query t0+i."""
    m = np.full((3, 128, 256), NEG, dtype=np.float32)
    for s in range(3):
        for r in range(128):
            lo = s * 128 + r - 128
            hi = s * 128 + r - 65
            lo_c = max(lo, 0)
            hi_c = min(hi, 255)
            if lo_c <= hi_c:
                m[s, r, lo_c:hi_c + 1] = 0.0
    return np.concatenate([m, m], axis=2)


def prep_inputs(x, Wq, Wk, Wv, Wo, last_k_init, last_v_init):
    """Shard + pre-transpose full inputs into 8 per-core input maps."""
    mask = make_mask()
    in_maps = []
    for core in range(NCORES):
        b, g = divmod(core, G)
        sl = slice(g * DG, (g + 1) * DG)
        lk = last_k_init[:, g * HPG:(g + 1) * HPG, :]   # [63, 4, 64]
        lv = last_v_init[:, g * HPG:(g + 1) * HPG, :]
        kcg = np.zeros((DG, 128), dtype=np.float32)
        kcg[:, 65:128] = lk.reshape(W - 1, DG).T
        vcg = np.zeros((128, HPG * (DH + 1)), dtype=np.float32)
        for h in range(HPG):
            vcg[65:128, h * (DH + 1):h * (DH + 1) + DH] = lv[:, h, :]
            vcg[65:128, h * (DH + 1) + DH] = 1.0
        in_maps.append({
            "xT": np.ascontiguousarray(x[b].T),
            "wqT": np.ascontiguousarray(Wq[sl, :].T),
            "wkT": np.ascontiguousarray(Wk[sl, :].T),
            "wvT": np.ascontiguousarray(Wv[sl, :].T),
            "woT": np.ascontiguousarray(Wo[:, sl].T),
            "kc": kcg,
            "vc": vcg,
            "onesr": np.ones((128, 32), dtype=np.float32),
            "onesf": np.ones((1, 64), dtype=np.float32),
            "mask": mask,
        })
    return in_maps


_built = None


def kernel(x, Wq, Wk, Wv, Wo, last_k_init, last_v_init) -> np.ndarray:
    global _built
    x = np.asarray(x, dtype=np.float32)
    args = [np.asarray(a, dtype=np.float32)
            for a in (Wq, Wk, Wv, Wo, last_k_init, last_v_init)]
    in_maps = prep_inputs(x, *args)
    if _built is None:
        _built = build()
    r = bass_utils.run_bass_kernel_spmd(
        _built, in_maps, core_ids=list(range(NCORES)))
    out = np.zeros((B, L, D), dtype=np.float32)
    for core in range(NCORES):
        b = core // G
        out[b] += r.results[core]["y"]
    return out



# revision 6
# speedup vs baseline: 1.3074x; 1.3074x over previous
"""Banded multi-head attention (B=2, L=1024, D=1024, H=16, band W=64) on 8
Trainium2 NeuronCores.

Sharding: core = (batch b, head-group g) with 2 batches x 4 head groups of 4
heads each.  Each core computes q/k/v projections for its group, the banded
attention for its 4 heads, and a partial output projection through its slice
of Wo.  Host sums the 4 partial outputs per batch.

Schedule (v2, bf16):
- All matmul operands are bf16 (psum accumulation stays fp32): halves HBM
  traffic and keeps the PE at 1 cycle/row.  Verified numerically: bf16
  operands give ~5e-3 relative error vs the 2e-2 gate.
- Band attention runs on 128-key chunks: for key chunk c (keys 128(c-1)..
  128c-1 in padded coordinates), the in-band queries span at most 191
  columns, so scores for a head pair live in one [128, 2, <=191] psum tile.
  The band mask in (key-row r, query-col j) coordinates is position
  independent (in-band iff 0 <= j - r <= 63), so a single [128, 2, 191]
  mask tile serves every chunk including the cache chunk c=0, whose zero
  padded rows fall outside the band automatically.
- Per (head, T-half) softmax denominators ride along as a ones-column in V;
  1/d comes from vector.reciprocal, is partition-broadcast on GPSIMD, and
  scales the attention output on DVE.  No Ln/Exp round trip, no broadcast
  matmul on the PE.
- Engine balance: Act does exp + q/k psum->sbuf copies (+ some y copies),
  DVE does half the mask stt, reciprocal and the normalizing multiplies,
  GPSIMD (otherwise idle) does the other half of the stt, V copies,
  partition broadcasts and most y copies.
- DMA count is minimized (shared HWDGE is ~0.6us per transfer): Wq/Wk/Wv
  ship as one fused [D, 3*DG] tensor (8 chunk DMAs), x as 16 half-row
  chunks, y as 8 [128, 1024] bf16 stores.
"""
import numpy as np
import ml_dtypes

import concourse.bacc as bacc
import concourse.mybir as mybir
import concourse.tile as tile
from concourse import bass_utils

B, L, D, H, W = 2, 1024, 1024, 16, 64
DH = D // H           # 64
G = 4                 # head groups
HPG = H // G          # 4 heads per group
DG = D // G           # 256 dims per group
NCORES = 8

F32 = mybir.dt.float32
BF16 = mybir.dt.bfloat16
NEG = -1.0e30
EXPF = mybir.ActivationFunctionType.Exp
SCALE = float(DH) ** -0.5

NCHUNK = 9            # key chunks: c=0 cache block, 1..8 token chunks
SPAN = 191            # max in-band query span per key chunk
KTW = 128 + L         # kT padded width per head
VSLOT = DH + 1        # 65 cols per (slot, head): 64 v dims + ones
VROW = HPG * VSLOT    # 260 cols per slot


def _pin_exp_table(arch: str):
    """Resolve Copy/Exp/Identity only to the natural_log_exp_and_others
    act-func set so exactly one table load is emitted (alternating per-
    function table swaps wedge the device)."""
    import concourse.hw_specs as hw_specs
    tables = hw_specs.get_activation_tables(arch)   # cached, mutable
    drop = {EXPF, mybir.ActivationFunctionType.Copy,
            mybir.ActivationFunctionType.Identity}
    assert "natural_log_exp_and_others" in tables
    for name, funcs in tables.items():
        if name != "natural_log_exp_and_others":
            funcs -= drop


def _chunk_qspan(c):
    """Query range [qlo, qhi) covered by key chunk c, plus the mask column
    offset jlo (j = q - 128*(c-1))."""
    qlo = max(0, 128 * c - 128)
    qhi = min(L, 128 * c + 63)
    return qlo, qhi, qlo - (128 * c - 128)


def build(repeat: int = 1, loop_n: int = 0):
    nc = bacc.Bacc("TRN2", target_bir_lowering=False, debug=False)
    _pin_exp_table(nc.m.arch)

    xT = nc.dram_tensor("xT", [D, L], BF16, kind="ExternalInput")
    wqkv = nc.dram_tensor("wqkv", [D, 3 * DG], BF16, kind="ExternalInput")
    woT = nc.dram_tensor("woT", [DG, D], BF16, kind="ExternalInput")
    kc = nc.dram_tensor("kc", [DH, HPG * 128], BF16, kind="ExternalInput")
    vc = nc.dram_tensor("vc", [128, VROW], BF16, kind="ExternalInput")
    onesr = nc.dram_tensor("onesr", [128, 32], BF16, kind="ExternalInput")
    maskd = nc.dram_tensor("mask", [128, 2 * SPAN], F32, kind="ExternalInput")
    y = nc.dram_tensor("y", [L, D], BF16, kind="ExternalOutput")

    with tile.TileContext(nc) as tc:
        with tc.tile_pool(name="res", bufs=1) as res, \
             tc.tile_pool(name="epool", bufs=20) as epool, \
             tc.tile_pool(name="rcpool", bufs=4) as rcpool, \
             tc.tile_pool(name="bcpool", bufs=4) as bcpool, \
             tc.tile_pool(name="ypool", bufs=3) as ypool, \
             tc.tile_pool(name="ps", bufs=8, space="PSUM") as psp:

            # ---- resident SBUF tensors ----------------------------------
            xk = [res.tile([128, L], BF16, tag=f"xk{k}", name=f"xk{k}")
                  for k in range(8)]
            wk3 = [res.tile([128, 3 * DG], BF16, tag=f"w3{k}", name=f"w3{k}")
                   for k in range(8)]
            wo_sb = [res.tile([128, D], BF16, tag=f"wo{m}", name=f"wo{m}")
                     for m in range(2)]
            qT = res.tile([64, HPG * L], BF16, tag="qT", name="qT")
            kT = res.tile([64, HPG * KTW], BF16, tag="kT", name="kT")
            v_sb = res.tile([128, NCHUNK * VROW], BF16, tag="v", name="v_sb")
            mask_sb = res.tile([128, 2, SPAN], F32, tag="mask", name="mask_sb")
            oT = [res.tile([128, L], BF16, tag=f"oT{m}", name=f"oT{m}")
                  for m in range(2)]

            def wslice(k, which):
                return wk3[k][:, which * DG:(which + 1) * DG]

            def emit_qk_group(which, dst, off, wid, m, n):
                """Project x through Wq/Wk chunk column block m for token
                half n; write per-head slices of dst ([64, HPG*wid])."""
                pt = psp.tile([128, 512], F32, tag="ps", name="pj")
                for k in range(8):
                    nc.tensor.matmul(
                        pt[:],
                        wslice(k, which)[:, m * 128:(m + 1) * 128],
                        xk[k][:, n * 512:(n + 1) * 512],
                        start=(k == 0), stop=(k == 7),
                    )
                for hh in range(2):
                    h = 2 * m + hh
                    dsl = dst[:, h * wid + off + n * 512:
                              h * wid + off + n * 512 + 512]
                    if hh == 0:
                        nc.scalar.copy(dsl, pt[0:64, :])
                    else:
                        nc.vector.tensor_copy(dsl, pt[64:128, :])

            def emit_v(t):
                pv = psp.tile([128, 512], F32, tag="ps", name="pjv")
                for k in range(8):
                    nc.tensor.matmul(
                        pv[:, 0:DG],
                        xk[k][:, t * 128:(t + 1) * 128],
                        wslice(k, 2),
                        start=(k == 0), stop=(k == 7),
                    )
                si = t + 1
                dst = v_sb[:, si * VROW:(si + 1) * VROW].rearrange(
                    "p (h c) -> p h c", c=VSLOT)[:, :, 0:DH]
                nc.scalar.copy(
                    dst, pv[:, 0:DG].rearrange("p (h c) -> p h c", c=DH))

            def emit_scores(m, c):
                """Scores for head pair m, key chunk c -> exp'd bf16 tile."""
                qlo, qhi, jlo = _chunk_qspan(c)
                span = qhi - qlo
                st = psp.tile([128, 2, SPAN], F32, tag="ps", name="st")
                for hh in range(2):
                    h = 2 * m + hh
                    nc.tensor.matmul(
                        st[:, hh, 0:span],
                        kT[:, h * KTW + 128 * c: h * KTW + 128 * c + 128],
                        qT[:, h * L + qlo: h * L + qhi],
                        start=True, stop=True,
                    )
                nc.vector.scalar_tensor_tensor(
                    st[:, :, 0:span], st[:, :, 0:span], SCALE,
                    mask_sb[:, :, jlo:jlo + span],
                    mybir.AluOpType.mult, mybir.AluOpType.add,
                )
                e = epool.tile([128, 2, SPAN], BF16, tag="e", name="e")
                nc.scalar.activation(e[:, :, 0:span], st[:, :, 0:span], EXPF)
                return e

            def emit_attnv(m, T, es):
                """Attention @ V for head pair m, query half T using the
                per-chunk exp tiles es[c]; returns psum tiles per head.

                PSUM pending-zero semantics: start=True marks the whole 2KB
                bank pending; any write to a pending byte zeroes it first.
                So per chunk emit the accumulate piece (into the range the
                PREVIOUS chunk's start just cleared) before this chunk's own
                start piece, which re-marks the bank."""
                base = 512 * T
                ops = []
                for hh in range(2):
                    hg = 2 * m + hh
                    op = psp.tile([65, 512], F32, tag="ps", name="op")
                    mms = []
                    prev_hi = base
                    for c in range(4 * T, 4 * T + 5):
                        qlo, qhi, _ = _chunk_qspan(c)
                        olo = max(qlo, base)
                        ohi = min(qhi, base + 512)
                        if olo < prev_hi:          # accumulate piece
                            mms.append((c, qlo, olo, prev_hi, False))
                        if ohi > prev_hi:          # fresh (start) piece
                            mms.append((c, qlo, prev_hi, ohi, True))
                        prev_hi = max(prev_hi, ohi)
                    for i, (c, qlo, lo, hi, st) in enumerate(mms):
                        nc.tensor.matmul(
                            op[:, lo - base: hi - base],
                            v_sb[:, c * VROW + hg * VSLOT:
                                 c * VROW + hg * VSLOT + VSLOT],
                            es[c][:, hh, lo - qlo: hi - qlo],
                            start=st, stop=(i == len(mms) - 1),
                            skip_group_check=True,
                        )
                    ops.append(op)
                return ops

            def emit_norm(m, T, ops):
                """Scale attention outputs by 1/denominator into oT."""
                for hh in range(2):
                    op = ops[hh]
                    rc = rcpool.tile([1, 512], F32, tag="rc", name="rc")
                    nc.vector.reciprocal(rc[:], op[64:65, :])
                    bc = bcpool.tile([64, 512], F32, tag="bc", name="bc")
                    nc.gpsimd.partition_broadcast(bc[:], rc[:])
                    nc.vector.tensor_mul(
                        oT[m][hh * 64:(hh + 1) * 64, T * 512:(T + 1) * 512],
                        op[0:64, :], bc[:])

            def emit_oproj(t):
                ysb = ypool.tile([128, 1024], BF16, tag="y", name="ysb")
                for n2 in range(2):
                    yp = psp.tile([128, 512], F32, tag="ps", name="yp")
                    for m in range(2):
                        nc.tensor.matmul(
                            yp[:],
                            oT[m][:, t * 128:(t + 1) * 128],
                            wo_sb[m][:, n2 * 512:(n2 + 1) * 512],
                            start=(m == 0), stop=(m == 1),
                        )
                    dsl = ysb[:, n2 * 512:(n2 + 1) * 512]
                    nc.scalar.copy(dsl, yp[:])
                nc.sync.dma_start(
                    y.ap()[t * 128:(t + 1) * 128, :], ysb[:])

            import contextlib

            def rep_ctx():
                if loop_n:
                    return tc.For_i(0, loop_n, 1,
                                    hint_engines=(mybir.EngineType.PE,
                                                  mybir.EngineType.Activation,
                                                  mybir.EngineType.DVE,
                                                  mybir.EngineType.Pool,
                                                  mybir.EngineType.SP))
                return contextlib.nullcontext()

            with rep_ctx():
              for rep in range(repeat):
                # ---- input DMAs: K-chunk streaming, x first-half first ---
                for k in range(8):
                    nc.sync.dma_start(wk3[k][:],
                                      wqkv.ap()[k * 128:(k + 1) * 128, :])
                    nc.sync.dma_start(
                        xk[k][:, 0:512],
                        xT.ap()[k * 128:(k + 1) * 128, 0:512])
                nc.sync.dma_start(
                    kT[:].rearrange("p (h c) -> p h c", c=KTW)[:, :, 0:128],
                    kc.ap().rearrange("p (h c) -> p h c", c=128),
                )
                nc.sync.dma_start(v_sb[:, 0:VROW], vc.ap())
                ones_cols = v_sb[:, VROW:].rearrange(
                    "p (n c) -> p n c", c=VSLOT)[:, :, DH:DH + 1]
                nc.sync.dma_start(
                    ones_cols, onesr.ap()[:, 0:32].unsqueeze(2))
                nc.sync.dma_start(
                    mask_sb[:], maskd.ap().rearrange(
                        "p (s n) -> p s n", s=2))
                for m in range(2):
                    nc.sync.dma_start(wo_sb[m][:],
                                      woT.ap()[m * 128:(m + 1) * 128, :])
                for k in range(8):
                    nc.sync.dma_start(
                        xk[k][:, 512:1024],
                        xT.ap()[k * 128:(k + 1) * 128, 512:1024])

                # ---- projections -----------------------------------------
                for m in range(2):
                    emit_qk_group(0, qT, 0, L, m, 0)
                    emit_qk_group(1, kT, 128, KTW, m, 0)
                emit_v(0)
                emit_v(1)
                for m in range(2):
                    emit_qk_group(0, qT, 0, L, m, 1)
                    emit_qk_group(1, kT, 128, KTW, m, 1)
                emit_v(2)
                emit_v(3)

                # ---- attention -------------------------------------------
                for m in range(2):
                    es = {}
                    for c in range(5):
                        es[c] = emit_scores(m, c)
                        if m == 0 and c >= 1 and c <= 4:
                            emit_v(c + 3)
                    ops0 = emit_attnv(m, 0, es)
                    for c in range(5, NCHUNK):
                        es[c] = emit_scores(m, c)
                    emit_norm(m, 0, ops0)
                    ops1 = emit_attnv(m, 1, es)
                    emit_norm(m, 1, ops1)

                # ---- output projection -----------------------------------
                for t in range(8):
                    emit_oproj(t)

    nc.compile()
    return nc


def make_mask() -> np.ndarray:
    """[128, 2*SPAN] additive mask in chunk coordinates: key row r, query
    col j (query index q = 128*(c-1) + j); in-band iff 0 <= j - r <= 63.
    Doubled for the two heads sharing one score tile."""
    r = np.arange(128)[:, None]
    j = np.arange(SPAN)[None, :]
    m = np.where((j - r >= 0) & (j - r <= 63), 0.0, NEG).astype(np.float32)
    return np.concatenate([m, m], axis=1)


def prep_inputs(x, Wq, Wk, Wv, Wo, last_k_init, last_v_init):
    """Shard + pre-transpose full inputs into 8 per-core input maps."""
    bf = ml_dtypes.bfloat16
    mask = make_mask()
    in_maps = []
    for core in range(NCORES):
        b, g = divmod(core, G)
        sl = slice(g * DG, (g + 1) * DG)
        lk = last_k_init[:, g * HPG:(g + 1) * HPG, :]   # [63, 4, 64]
        lv = last_v_init[:, g * HPG:(g + 1) * HPG, :]
        # cache K block per head: [64, 128] with cols 0..64 zero,
        # 65..127 = keys -63..-1; stacked [64, 4*128]
        kcg = np.zeros((DH, HPG * 128), dtype=np.float32)
        for h in range(HPG):
            kcg[:, h * 128 + 65: h * 128 + 128] = lk[:, h, :].T
        vcg = np.zeros((128, VROW), dtype=np.float32)
        for h in range(HPG):
            vcg[65:128, h * VSLOT:h * VSLOT + DH] = lv[:, h, :]
            vcg[65:128, h * VSLOT + DH] = 1.0
        wqkv = np.concatenate(
            [Wq[sl, :].T, Wk[sl, :].T, Wv[sl, :].T], axis=1)  # [D, 3*DG]
        in_maps.append({
            "xT": np.ascontiguousarray(x[b].T).astype(bf),
            "wqkv": np.ascontiguousarray(wqkv).astype(bf),
            "woT": np.ascontiguousarray(Wo[:, sl].T).astype(bf),
            "kc": kcg.astype(bf),
            "vc": vcg.astype(bf),
            "onesr": np.ones((128, 32), dtype=np.float32).astype(bf),
            "mask": mask,
        })
    return in_maps


_built = None


def kernel(x, Wq, Wk, Wv, Wo, last_k_init, last_v_init) -> np.ndarray:
    global _built
    x = np.asarray(x, dtype=np.float32)
    args = [np.asarray(a, dtype=np.float32)
            for a in (Wq, Wk, Wv, Wo, last_k_init, last_v_init)]
    in_maps = prep_inputs(x, *args)
    if _built is None:
        _built = build()
    r = bass_utils.run_bass_kernel_spmd(
        _built, in_maps, core_ids=list(range(NCORES)))
    out = np.zeros((B, L, D), dtype=np.float32)
    for core in range(NCORES):
        b = core // G
        out[b] += np.asarray(r.results[core]["y"], dtype=np.float32)
    return out
